# revision 25
# baseline (speedup 1.0000x reference)
"""VQ codebook quantization kernel for Trainium2 (8 NeuronCores, data-parallel).

Strategy (quad-balanced "pack" pipeline):
  - Shard the 16384 code rows across 8 cores (2048 rows each); replicate the
    codebook. argmin_k ||x - c_k||^2 == argmax_k (x.c_k - 0.5||c_k||^2).
  - PE: cross[n,k] = codes.codebook^T via a single float32r matmul pass
    (fp32 data at bf16 PE rate, ~11-bit effective product mantissa),
    16 n-tiles x 2 k-halves of PSUM (128, 2048).
  - ACT pass 1 (all half-tiles): q = Copy(psum * 2^25 + C2), C2 = 1.5*2^47.
    The fp32 ulp at C2 is 2^24, so q = C2 + 2^24 * round(2*cross): the score
    is snapped onto a 2^24 grid (0.5 in score units). Frees PSUM quickly.
  - Pack: packed = (q - C2) + combo[k], where combo[k] = M_k*2^24 +
    4096*(4095-k) and M_k = round(2*(OFFB - 0.5||c_k||^2)). Every
    intermediate is a multiple of 4096 below 2^35, so fp32 adds are EXACT:
    packed = 2^24*(round(2*cross) + M_k) + 4096*(4095-k).
    Ordering by packed == ordering by (biased score quantized to 0.5, then
    lower k), and bits [12..23] hold the codebook index. Values distinct.
    Routing: ~1/3 of half-tiles pack on DVE via one scalar_tensor_tensor
    ((q + -C2) + combo); the rest on GPSIMD (ACT pass 2 subtracts C2
    in-place, then a plain gpsimd tensor_add). This balances ACT/GPSIMD/DVE
    against the PE roofline (~109 us/core).
  - DVE: one max8 per (128,2048) half-tile -> top-8 packed values = top-8
    (approx score, index) candidates per half. No find_index pass needed.
  - Host: decode; rows whose top-2 bucket gap < tau (2.0 score units, vs
    total approx error < ~0.6 and typical top-2 gaps ~15) are rescored
    exactly (float64) against their 16 candidates. Sanity range checks
    guard the packing; exact host fallback on any violation/device failure.
"""

import numpy as np

N_CORES = 8
K = 4096            # codebook entries
D = 512             # embedding dim
ROWS_PER_CORE = 2048
N_TILES = ROWS_PER_CORE // 128   # 16
HALF = 2048         # k entries per PSUM half-tile

SCALE = float(2 ** 25)      # ACT scale: score -> 2^24-grid via C2 snap
C2 = float(3 * 2 ** 46)     # 1.5 * 2^47; ulp(C2) = 2^24
OFFB = 900.0                # bias shift keeping buckets in (0, 2048)
TAU_BUCKET = 4              # trust gap: 4 buckets = 2.0 score units

_CACHED = {}


def _build_nc(reps=1, mid_bufs=3, dve_route_mod=3):
    import concourse.tile as tile
    from concourse import bacc, mybir

    f32 = mybir.dt.float32
    f32r = mybir.dt.float32r
    alu = mybir.AluOpType

    nc = bacc.Bacc("TRN2", target_bir_lowering=False, debug=False)

    codesT = nc.dram_tensor("codesT", [D, ROWS_PER_CORE], f32r, kind="ExternalInput")
    cbT = nc.dram_tensor("cbT", [D, K], f32r, kind="ExternalInput")
    combo = nc.dram_tensor("combo", [128, K], f32, kind="ExternalInput")
    out_maxv = nc.dram_tensor("maxv", [N_TILES, 128, 2, 8], f32, kind="ExternalOutput")

    C = D // 128  # contraction chunks

    with tile.TileContext(nc) as tc:
        with (
            tc.tile_pool(name="weights", bufs=1) as wpool,
            tc.tile_pool(name="mid", bufs=mid_bufs) as mpool,
            tc.tile_pool(name="small", bufs=8) as spool,
            tc.tile_pool(name="psum", bufs=2, space="PSUM") as ppool,
        ):
            # per-chunk loads so the first tile's matmuls start early
            codesT_sb = []
            for c in range(C):
                t = wpool.tile([128, ROWS_PER_CORE], f32r, tag=f"codesT{c}")
                nc.sync.dma_start(t[:], codesT[c * 128:(c + 1) * 128, :])
                codesT_sb.append(t)
            cbT_sb = [[None] * C for _ in range(2)]
            for h in range(2):
                for c in range(C):
                    t = wpool.tile([128, HALF], f32r, tag=f"cbT{h}_{c}")
                    nc.sync.dma_start(
                        t[:],
                        cbT[c * 128:(c + 1) * 128, h * HALF:(h + 1) * HALF],
                    )
                    cbT_sb[h][c] = t
            combo_t = wpool.tile([128, K], f32, tag="combo")
            nc.sync.dma_start(combo_t[:], combo[:])
            combo_sb = [combo_t]

            def body():
                for t in range(N_TILES):
                    for h in range(2):
                        idx = t * 2 + h
                        psum = ppool.tile([128, HALF], f32)
                        for c in range(C):
                            lhsT = codesT_sb[c][:, t * 128:(t + 1) * 128]
                            for j in range(HALF // 512):
                                nc.tensor.matmul(
                                    psum[:, j * 512:(j + 1) * 512],
                                    lhsT,
                                    cbT_sb[h][c][:, j * 512:(j + 1) * 512],
                                    start=(c == 0),
                                    stop=(c == C - 1),
                                )
                        # snap: q = C2 + 2^24*round(2*cross); frees PSUM
                        q_sb = mpool.tile([128, HALF], f32, tag="q")
                        nc.scalar.activation(
                            q_sb[:], psum[:],
                            mybir.ActivationFunctionType.Copy,
                            bias=C2, scale=SCALE,
                        )
                        packed = mpool.tile([128, HALF], f32, tag="packed")
                        if idx % dve_route_mod == dve_route_mod - 1 or idx >= 30:
                            # DVE route: fused (q + -C2) + combo
                            nc.vector.scalar_tensor_tensor(
                                packed[:], q_sb[:], -C2,
                                combo_sb[0][:, h * HALF:(h + 1) * HALF],
                                op0=alu.add, op1=alu.add,
                            )
                        else:
                            # ACT pass 2 (in-place subtract), then GPSIMD add
                            nc.scalar.activation(
                                q_sb[:], q_sb[:],
                                mybir.ActivationFunctionType.Copy,
                                bias=-C2, scale=1.0,
                            )
                            nc.gpsimd.tensor_add(
                                packed[:], q_sb[:],
                                combo_sb[0][:, h * HALF:(h + 1) * HALF],
                            )
                        maxv_sb = spool.tile([128, 8], f32, tag="maxv")
                        nc.vector.max(maxv_sb[:], packed[:])
                        nc.sync.dma_start(out_maxv[t, :, h, :], maxv_sb[:])

            if reps == 1:
                body()
            else:
                with tc.For_i(0, reps, 1):
                    body()
    nc.compile()
    return nc


def _get_nc():
    if "nc" not in _CACHED:
        _CACHED["nc"] = _build_nc()
    return _CACHED["nc"]


def _make_combo(cb_sq):
    """combo[p, k] = M_k*2^24 + 4096*(4095-k)."""
    M = np.rint(2.0 * (OFFB - 0.5 * cb_sq.astype(np.float64)))  # (K,)
    iota = 4095.0 - np.arange(K, dtype=np.float64)
    vals = M * float(2 ** 24) + 4096.0 * iota
    assert vals.min() > 0 and vals.max() < 2 ** 35
    v32 = vals.astype(np.float32)
    assert np.all(v32.astype(np.float64) == vals), "combo not fp32-exact"
    return np.ascontiguousarray(np.broadcast_to(v32[None, :], (128, K)))


def kernel(codebook, codes):
    from concourse.bass_utils import run_bass_kernel_spmd

    codebook = np.ascontiguousarray(np.asarray(codebook, dtype=np.float32))
    codes = np.ascontiguousarray(np.asarray(codes, dtype=np.float32))
    B, T, _ = codes.shape  # (8, 2048, 512)
    flat = codes.reshape(B * T, D)

    cbT = np.ascontiguousarray(codebook.T)  # (512, 4096)
    cb_sq = np.einsum("kd,kd->k", codebook, codebook, dtype=np.float32)
    try:
        combo = _make_combo(cb_sq)
    except AssertionError:
        # codebook statistics outside packing assumptions
        return _full_host_fallback(codebook, codes)

    in_maps = []
    for i in range(N_CORES):
        rows = flat[i * ROWS_PER_CORE:(i + 1) * ROWS_PER_CORE]
        in_maps.append(
            {
                "codesT": np.ascontiguousarray(rows.T),
                "cbT": cbT,
                "combo": combo,
            }
        )

    results = None
    for _attempt in range(3):
        try:
            nc = _get_nc()
            results = run_bass_kernel_spmd(
                nc, in_maps, core_ids=list(range(N_CORES))
            ).results
            break
        except Exception:
            _CACHED.clear()
    if results is None:
        return _full_host_fallback(codebook, codes)

    # ---- host-side decode + candidate resolution ----
    packed = np.empty((B * T, 16), dtype=np.int64)
    for i, res in enumerate(results):
        v = res["maxv"].reshape(ROWS_PER_CORE, 16)         # (t*128+p, h*8+e)
        packed[i * ROWS_PER_CORE:(i + 1) * ROWS_PER_CORE] = v.astype(np.int64)

    bucket = packed >> 24
    ii = (packed >> 12) & 4095
    if (
        packed.min() <= 0
        or bucket.min() <= 0
        or bucket.max() >= 2048
        or (packed & 4095).any()
    ):
        return _full_host_fallback(codebook, codes)

    kidx = 4095 - ii                                       # (N, 8)

    # rank candidates by (bucket desc, k asc)
    order = np.lexsort((kidx, -bucket), axis=1)
    b_sorted = np.take_along_axis(bucket, order, axis=1)
    k_sorted = np.take_along_axis(kidx, order, axis=1)
    gap = b_sorted[:, 0] - b_sorted[:, 1]

    quant_id = k_sorted[:, 0].copy()

    amb = np.nonzero(gap < TAU_BUCKET)[0]
    if amb.size:
        cand = kidx[amb]                          # (A, 16)
        x = flat[amb].astype(np.float64)
        cvec = codebook.astype(np.float64)[cand]  # (A, 16, 512)
        cross = np.einsum("ad,acd->ac", x, cvec)
        cb2 = np.einsum("kd,kd->k", codebook.astype(np.float64),
                        codebook.astype(np.float64))
        d2 = cb2[cand] - 2.0 * cross
        ordc = np.lexsort((cand, d2), axis=-1)
        quant_id[amb] = np.take_along_axis(cand, ordc[:, :1], axis=1)[:, 0]

    quant_id = quant_id.astype(np.int32)
    quant_codes = codebook[quant_id].reshape(B, T, D)
    return quant_codes, quant_id.reshape(B, T)


def _full_host_fallback(codebook, codes):
    """Exact CPU fallback (device failure / sanity violation; never expected).

    f32 GEMM for candidate search, f64 rescore of the top-8 for exactness.
    """
    B, T, _ = codes.shape
    flat = codes.reshape(-1, D)
    cb = codebook
    cb2_64 = np.einsum(
        "kd,kd->k", cb.astype(np.float64), cb.astype(np.float64)
    )
    ids = np.empty(flat.shape[0], dtype=np.int32)
    for i in range(0, flat.shape[0], 2048):
        blk = flat[i:i + 2048]
        d2 = cb2_64.astype(np.float32)[None, :] - 2.0 * (blk @ cb.T)
        cand = np.argpartition(d2, 8, axis=1)[:, :8].astype(np.int64)
        cross = np.einsum(
            "ad,acd->ac", blk.astype(np.float64), cb.astype(np.float64)[cand]
        )
        d2c = cb2_64[cand] - 2.0 * cross
        ordc = np.lexsort((cand, d2c), axis=-1)
        ids[i:i + 2048] = np.take_along_axis(cand, ordc[:, :1], axis=1)[:, 0]
    return codebook[ids].reshape(B, T, D), ids.reshape(B, T)


# revision 29
# speedup vs baseline: 1.0663x; 1.0663x over previous
"""VQ codebook quantization kernel for Trainium2 (8 NeuronCores, data-parallel).

Strategy (quad-balanced "pack" pipeline):
  - Shard the 16384 code rows across 8 cores (2048 rows each); replicate the
    codebook. argmin_k ||x - c_k||^2 == argmax_k (x.c_k - 0.5||c_k||^2).
  - PE: cross[n,k] = codes.codebook^T via a single float32r matmul pass
    (fp32 data at bf16 PE rate, ~11-bit effective product mantissa),
    16 n-tiles x 2 k-halves of PSUM (128, 2048).
  - ACT pass 1 (all half-tiles): q = Copy(psum * 2^25 + C2), C2 = 1.5*2^47.
    The fp32 ulp at C2 is 2^24, so q = C2 + 2^24 * round(2*cross): the score
    is snapped onto a 2^24 grid (0.5 in score units). Frees PSUM quickly.
  - Pack: packed = (q - C2) + combo[k], where combo[k] = M_k*2^24 +
    4096*(4095-k) and M_k = round(2*(OFFB - 0.5||c_k||^2)). Every
    intermediate is a multiple of 4096 below 2^35, so fp32 adds are EXACT:
    packed = 2^24*(round(2*cross) + M_k) + 4096*(4095-k).
    Ordering by packed == ordering by (biased score quantized to 0.5, then
    lower k), and bits [12..23] hold the codebook index. Values distinct.
    Routing: ~1/3 of half-tiles pack on DVE via one scalar_tensor_tensor
    ((q + -C2) + combo); the rest on GPSIMD (ACT pass 2 subtracts C2
    in-place, then a plain gpsimd tensor_add). This balances ACT/GPSIMD/DVE
    against the PE roofline (~109 us/core).
  - DVE: one max8 per (128,2048) half-tile -> top-8 packed values = top-8
    (approx score, index) candidates per half. No find_index pass needed.
  - Host: decode; rows whose top-2 bucket gap < tau (2.0 score units, vs
    total approx error < ~0.6 and typical top-2 gaps ~15) are rescored
    exactly (float64) against their 16 candidates. Sanity range checks
    guard the packing; exact host fallback on any violation/device failure.
"""

import numpy as np

N_CORES = 8
K = 4096            # codebook entries
D = 512             # embedding dim
ROWS_PER_CORE = 2048
N_TILES = ROWS_PER_CORE // 128   # 16
HALF = 2048         # k entries per PSUM half-tile

SCALE = float(2 ** 25)      # ACT scale: score -> 2^24-grid via C2 snap
C2 = float(3 * 2 ** 46)     # 1.5 * 2^47; ulp(C2) = 2^24
OFFB = 900.0                # bias shift keeping buckets in (0, 2048)
TAU_BUCKET = 4              # trust gap: 4 buckets = 2.0 score units
USE_BF16 = True             # bf16 matmul inputs (faster PE stream, ~same error)

_CACHED = {}


def _build_nc(reps=1, mid_bufs=3, dve_route_mod=3):
    import concourse.tile as tile
    from concourse import bacc, mybir

    f32 = mybir.dt.float32
    mm_dt = mybir.dt.bfloat16 if USE_BF16 else mybir.dt.float32r
    alu = mybir.AluOpType

    nc = bacc.Bacc("TRN2", target_bir_lowering=False, debug=False)

    codesT = nc.dram_tensor("codesT", [D, ROWS_PER_CORE], mm_dt, kind="ExternalInput")
    cbT = nc.dram_tensor("cbT", [D, K], mm_dt, kind="ExternalInput")
    combo = nc.dram_tensor("combo", [128, K], f32, kind="ExternalInput")
    out_maxv = nc.dram_tensor("maxv", [N_TILES, 128, 2, 8], f32, kind="ExternalOutput")

    C = D // 128  # contraction chunks

    with tile.TileContext(nc) as tc:
        with (
            tc.tile_pool(name="weights", bufs=1) as wpool,
            tc.tile_pool(name="mid", bufs=mid_bufs) as mpool,
            tc.tile_pool(name="small", bufs=8) as spool,
            tc.tile_pool(name="psum", bufs=2, space="PSUM") as ppool,
        ):
            # per-chunk loads so the first tile's matmuls start early
            codesT_sb = []
            for c in range(C):
                t = wpool.tile([128, ROWS_PER_CORE], mm_dt, tag=f"codesT{c}")
                nc.sync.dma_start(t[:], codesT[c * 128:(c + 1) * 128, :])
                codesT_sb.append(t)
            cbT_sb = [[None] * C for _ in range(2)]
            for h in range(2):
                for c in range(C):
                    t = wpool.tile([128, HALF], mm_dt, tag=f"cbT{h}_{c}")
                    nc.sync.dma_start(
                        t[:],
                        cbT[c * 128:(c + 1) * 128, h * HALF:(h + 1) * HALF],
                    )
                    cbT_sb[h][c] = t
            combo_t = wpool.tile([128, K], f32, tag="combo")
            nc.sync.dma_start(combo_t[:], combo[:])
            combo_sb = [combo_t]

            def body():
                for t in range(N_TILES):
                    for h in range(2):
                        idx = t * 2 + h
                        psum = ppool.tile([128, HALF], f32)
                        for c in range(C):
                            lhsT = codesT_sb[c][:, t * 128:(t + 1) * 128]
                            for j in range(HALF // 512):
                                nc.tensor.matmul(
                                    psum[:, j * 512:(j + 1) * 512],
                                    lhsT,
                                    cbT_sb[h][c][:, j * 512:(j + 1) * 512],
                                    start=(c == 0),
                                    stop=(c == C - 1),
                                )
                        # snap: q = C2 + 2^24*round(2*cross); frees PSUM
                        q_sb = mpool.tile([128, HALF], f32, tag="q")
                        nc.scalar.activation(
                            q_sb[:], psum[:],
                            mybir.ActivationFunctionType.Copy,
                            bias=C2, scale=SCALE,
                        )
                        packed = mpool.tile([128, HALF], f32, tag="packed")
                        if idx % dve_route_mod == dve_route_mod - 1 or idx >= 30:
                            # DVE route: fused (q + -C2) + combo
                            nc.vector.scalar_tensor_tensor(
                                packed[:], q_sb[:], -C2,
                                combo_sb[0][:, h * HALF:(h + 1) * HALF],
                                op0=alu.add, op1=alu.add,
                            )
                        else:
                            # ACT pass 2 (in-place subtract), then GPSIMD add
                            nc.scalar.activation(
                                q_sb[:], q_sb[:],
                                mybir.ActivationFunctionType.Copy,
                                bias=-C2, scale=1.0,
                            )
                            nc.gpsimd.tensor_add(
                                packed[:], q_sb[:],
                                combo_sb[0][:, h * HALF:(h + 1) * HALF],
                            )
                        maxv_sb = spool.tile([128, 8], f32, tag="maxv")
                        nc.vector.max(maxv_sb[:], packed[:])
                        nc.sync.dma_start(out_maxv[t, :, h, :], maxv_sb[:])

            if reps == 1:
                body()
            else:
                with tc.For_i(0, reps, 1):
                    body()
    nc.compile()
    return nc


def _get_nc():
    if "nc" not in _CACHED:
        _CACHED["nc"] = _build_nc()
    return _CACHED["nc"]


def _make_combo(cb_sq):
    """combo[p, k] = M_k*2^24 + 4096*(4095-k)."""
    M = np.rint(2.0 * (OFFB - 0.5 * cb_sq.astype(np.float64)))  # (K,)
    iota = 4095.0 - np.arange(K, dtype=np.float64)
    vals = M * float(2 ** 24) + 4096.0 * iota
    assert vals.min() > 0 and vals.max() < 2 ** 35
    v32 = vals.astype(np.float32)
    assert np.all(v32.astype(np.float64) == vals), "combo not fp32-exact"
    return np.ascontiguousarray(np.broadcast_to(v32[None, :], (128, K)))


def kernel(codebook, codes):
    from concourse.bass_utils import run_bass_kernel_spmd

    codebook = np.ascontiguousarray(np.asarray(codebook, dtype=np.float32))
    codes = np.ascontiguousarray(np.asarray(codes, dtype=np.float32))
    B, T, _ = codes.shape  # (8, 2048, 512)
    flat = codes.reshape(B * T, D)

    cbT = np.ascontiguousarray(codebook.T)  # (512, 4096)
    cb_sq = np.einsum("kd,kd->k", codebook, codebook, dtype=np.float32)
    try:
        combo = _make_combo(cb_sq)
    except AssertionError:
        # codebook statistics outside packing assumptions
        return _full_host_fallback(codebook, codes)

    if USE_BF16:
        import ml_dtypes
        cbT_dev = np.ascontiguousarray(cbT.astype(ml_dtypes.bfloat16))
    else:
        cbT_dev = cbT
    in_maps = []
    for i in range(N_CORES):
        rows = flat[i * ROWS_PER_CORE:(i + 1) * ROWS_PER_CORE]
        codesT_i = np.ascontiguousarray(rows.T)
        if USE_BF16:
            import ml_dtypes
            codesT_i = np.ascontiguousarray(codesT_i.astype(ml_dtypes.bfloat16))
        in_maps.append(
            {
                "codesT": codesT_i,
                "cbT": cbT_dev,
                "combo": combo,
            }
        )

    results = None
    for _attempt in range(3):
        try:
            nc = _get_nc()
            results = run_bass_kernel_spmd(
                nc, in_maps, core_ids=list(range(N_CORES))
            ).results
            break
        except Exception:
            _CACHED.clear()
    if results is None:
        return _full_host_fallback(codebook, codes)

    # ---- host-side decode + candidate resolution ----
    packed = np.empty((B * T, 16), dtype=np.int64)
    for i, res in enumerate(results):
        v = res["maxv"].reshape(ROWS_PER_CORE, 16)         # (t*128+p, h*8+e)
        packed[i * ROWS_PER_CORE:(i + 1) * ROWS_PER_CORE] = v.astype(np.int64)

    bucket = packed >> 24
    ii = (packed >> 12) & 4095
    if (
        packed.min() <= 0
        or bucket.min() <= 0
        or bucket.max() >= 2048
        or (packed & 4095).any()
    ):
        return _full_host_fallback(codebook, codes)

    kidx = 4095 - ii                                       # (N, 8)

    # rank candidates by (bucket desc, k asc)
    order = np.lexsort((kidx, -bucket), axis=1)
    b_sorted = np.take_along_axis(bucket, order, axis=1)
    k_sorted = np.take_along_axis(kidx, order, axis=1)
    gap = b_sorted[:, 0] - b_sorted[:, 1]

    quant_id = k_sorted[:, 0].copy()

    amb = np.nonzero(gap < TAU_BUCKET)[0]
    if amb.size:
        cand = kidx[amb]                          # (A, 16)
        x = flat[amb].astype(np.float64)
        cvec = codebook.astype(np.float64)[cand]  # (A, 16, 512)
        cross = np.einsum("ad,acd->ac", x, cvec)
        cb2 = np.einsum("kd,kd->k", codebook.astype(np.float64),
                        codebook.astype(np.float64))
        d2 = cb2[cand] - 2.0 * cross
        ordc = np.lexsort((cand, d2), axis=-1)
        quant_id[amb] = np.take_along_axis(cand, ordc[:, :1], axis=1)[:, 0]

    quant_id = quant_id.astype(np.int32)
    quant_codes = codebook[quant_id].reshape(B, T, D)
    return quant_codes, quant_id.reshape(B, T)


def _full_host_fallback(codebook, codes):
    """Exact CPU fallback (device failure / sanity violation; never expected).

    f32 GEMM for candidate search, f64 rescore of the top-8 for exactness.
    """
    B, T, _ = codes.shape
    flat = codes.reshape(-1, D)
    cb = codebook
    cb2_64 = np.einsum(
        "kd,kd->k", cb.astype(np.float64), cb.astype(np.float64)
    )
    ids = np.empty(flat.shape[0], dtype=np.int32)
    for i in range(0, flat.shape[0], 2048):
        blk = flat[i:i + 2048]
        d2 = cb2_64.astype(np.float32)[None, :] - 2.0 * (blk @ cb.T)
        cand = np.argpartition(d2, 8, axis=1)[:, :8].astype(np.int64)
        cross = np.einsum(
            "ad,acd->ac", blk.astype(np.float64), cb.astype(np.float64)[cand]
        )
        d2c = cb2_64[cand] - 2.0 * cross
        ordc = np.lexsort((cand, d2c), axis=-1)
        ids[i:i + 2048] = np.take_along_axis(cand, ordc[:, :1], axis=1)[:, 0]
    return codebook[ids].reshape(B, T, D), ids.reshape(B, T)
